# revision 2
# baseline (speedup 1.0000x reference)
"""Multi-head attention (B=8, S=1024, D=1024, H=16) on 8 TRN2 NeuronCores.

Sharding: data-parallel over batch — core b computes batch element b
end-to-end. No collectives.

v3 schedule: one software-pipelined "slot" loop over head pairs p; slot jt
runs scores(p, jt) [K=64 matmuls, row-group paired for PE concurrency],
exp on ScalarE, ctx-chain matmuls of pair p-1, and Q/K projection-chain
matmuls of pair p+1, so neither exp latency nor projection drains stall
the PE.

  - wq/wk are stored e-tile-major in DRAM (host-side permute) so pair 0's
    projections need only 4.5 MB of DMA before starting.
  - scores: K=64 matmuls, head h from SBUF partitions 0:64 and head h+1
    from 64:128, emitted alternately -> distinct PE row groups run
    concurrently (~1.7x measured).
  - softmax denominator: V_aug ones column; ctx chains give [65,512] with
    den in row 64; normalize = DVE reciprocal -> one gpsimd
    partition-broadcast per head (bf16) -> DVE multiplies.
  - PSUM: ps 2x[128,1024] (V proj / scores / out proj) + pchain 4x[*,512]
    (Q/K projection chains and ctx chains share one 4-bank ring, giving
    enough slack to hide drain/normalize latency) = 8 banks.
  - output DMA goes out on the Activation HWDGE queue so next iteration's
    input prefetch (SP queue) is not blocked behind it.
"""

import numpy as np
import ml_dtypes

import concourse.bass as bass
import concourse.mybir as mybir
import concourse.tile as tile
from concourse import bacc
from concourse.bass_utils import run_bass_kernel_spmd

BF = ml_dtypes.bfloat16

B, S, D, H = 8, 1024, 1024, 16
DK = D // H            # 64
P = 128
KT = D // P            # 8 contraction chunks
ET = D // P            # 8 e-tiles == head pairs
ST = S // P            # 8 s/j tiles
FREE = 512
NIH = S // FREE        # 2 i-halves
N_CORES = 8

F32 = mybir.dt.float32
BF16 = mybir.dt.bfloat16
EXP_SCALE = float(1.0 / np.sqrt(DK))


def build_nc(repeat: int = 1, stages: str = "v,qk,scores,ctx,out"):
    stage_set = set(stages.split(","))
    do_v = "v" in stage_set
    do_qk = "qk" in stage_set
    do_scores = do_qk and "scores" in stage_set
    do_ctx = do_scores and do_v and "ctx" in stage_set
    do_out = do_ctx and "out" in stage_set
    nc = bacc.Bacc("TRN2", target_bir_lowering=False, debug=False,
                   num_devices=N_CORES)

    xq_d = nc.dram_tensor("xq_t", [D, S], BF16, kind="ExternalInput")
    xk_d = nc.dram_tensor("xk_t", [D, S], BF16, kind="ExternalInput")
    xv_d = nc.dram_tensor("xv_t", [D, S], BF16, kind="ExternalInput")
    wq_d = nc.dram_tensor("wq_e", [D, D], BF16, kind="ExternalInput")
    wk_d = nc.dram_tensor("wk_e", [D, D], BF16, kind="ExternalInput")
    wv_d = nc.dram_tensor("wv_t", [D, D], BF16, kind="ExternalInput")
    wo_d = nc.dram_tensor("wo_t", [D, D], BF16, kind="ExternalInput")
    bq_d = nc.dram_tensor("bq_r", [P, ET], F32, kind="ExternalInput")
    bk_d = nc.dram_tensor("bk_r", [P, ET], F32, kind="ExternalInput")
    bvb_d = nc.dram_tensor("bvb", [P, D], BF16, kind="ExternalInput")
    bob_d = nc.dram_tensor("bob", [P, D], BF16, kind="ExternalInput")
    out_d = nc.dram_tensor("out", [S, D], F32, kind="ExternalOutput")

    with tile.TileContext(nc) as tc:
        with tc.tile_pool(name="xin", bufs=8) as xin, \
             tc.tile_pool(name="wqk", bufs=4) as wqk, \
             tc.tile_pool(name="wvo", bufs=8) as wvo, \
             tc.tile_pool(name="qk", bufs=2) as qk, \
             tc.tile_pool(name="att", bufs=18) as att, \
             tc.tile_pool(name="vau", bufs=1) as vau, \
             tc.tile_pool(name="ctx", bufs=1) as ctxp, \
             tc.tile_pool(name="outp", bufs=2) as outp, \
             tc.tile_pool(name="rbp", bufs=2) as rbp, \
             tc.tile_pool(name="cst", bufs=1) as cst, \
             tc.tile_pool(name="ps", bufs=2, space="PSUM") as ps, \
             tc.tile_pool(name="pchain", bufs=4, space="PSUM") as pch:

            bq_sb = cst.tile([P, ET], F32, name="bq_sb")
            bk_sb = cst.tile([P, ET], F32, name="bk_sb")
            bvb_sb = cst.tile([P, D], BF16, name="bvb_sb")
            bob_sb = cst.tile([P, D], BF16, name="bob_sb")

            vaug = [vau.tile([P, H, 2 * DK], BF16, tag=f"vaug{st}",
                             name=f"vaug{st}")
                    for st in range(ST)]
            # ones FIRST: the ctx matmul then puts the softmax denominator
            # on psum partitions 0:64 (reciprocal_approx_fast needs base 0)
            for st in range(ST):
                nc.gpsimd.memset(vaug[st][:, :, 0:DK], 1.0)

            def body():
                nc.sync.dma_start(out=bq_sb[:], in_=bq_d[:])
                nc.sync.dma_start(out=bk_sb[:], in_=bk_d[:])

                # DMA plan: QK pair-0 data first (et-major weights), then
                # V data, then remaining weight e-tiles, then wo.
                xq_sb, wq_sb, xk_sb, wk_sb, xv_sb, wv_sb = \
                    [], [], [], [], [], []
                if do_qk:
                    for dst, src, pool, tg in ((wq_sb, wq_d, wqk, "wq"),
                                               (xq_sb, xq_d, xin, "xq"),
                                               (wk_sb, wk_d, wqk, "wk"),
                                               (xk_sb, xk_d, xin, "xk")):
                        n0 = 1 if tg[0] == "w" else KT
                        for k in range(n0):
                            t = pool.tile([P, S], BF16, tag=tg,
                                          name=f"{tg}{k}")
                            nc.sync.dma_start(out=t[:],
                                              in_=src[k * P:(k + 1) * P, :])
                            dst.append(t)
                    # second e-tile of the weights right after pair-0 data
                    for dst, src, tg in ((wq_sb, wq_d, "wq"),
                                         (wk_sb, wk_d, "wk")):
                        t = wqk.tile([P, S], BF16, tag=tg, name=f"{tg}1")
                        nc.sync.dma_start(out=t[:], in_=src[P:2 * P, :])
                        dst.append(t)
                if do_v:
                    for k in range(KT):
                        tx = xin.tile([P, S], BF16, tag="xv", name=f"xv{k}")
                        nc.sync.dma_start(out=tx[:],
                                          in_=xv_d[k * P:(k + 1) * P, :])
                        xv_sb.append(tx)
                        tw = wvo.tile([P, S], BF16, tag="wvo", name=f"wv{k}")
                        nc.sync.dma_start(out=tw[:],
                                          in_=wv_d[k * P:(k + 1) * P, :])
                        wv_sb.append(tw)
                nc.sync.dma_start(out=bvb_sb[:], in_=bvb_d[:])
                nc.sync.dma_start(out=bob_sb[:], in_=bob_d[:])
                if do_qk:
                    for et in range(2, ET):
                        for dst, src, tg in ((wq_sb, wq_d, "wq"),
                                             (wk_sb, wk_d, "wk")):
                            t = wqk.tile([P, S], BF16, tag=tg,
                                         name=f"{tg}{et}")
                            nc.sync.dma_start(
                                out=t[:], in_=src[et * P:(et + 1) * P, :])
                            dst.append(t)
                wo_sb = []
                for k in range(KT if do_out else 0):
                    t = wvo.tile([P, S], BF16, tag="wvo", name=f"wo{k}")
                    nc.sync.dma_start(out=t[:], in_=wo_d[k * P:(k + 1) * P, :])
                    wo_sb.append(t)

                qt_sb = [None] * ET
                kt_sb = [None] * ET

                def proj_chain(p, which, ih):
                    """One [128,512] Q/K chain for pair p, half ih."""
                    x_sb, w_sb, b_sb, tg = (
                        (xq_sb, wq_sb, bq_sb, "qt") if which == "q"
                        else (xk_sb, wk_sb, bk_sb, "kt"))
                    dst = qt_sb if which == "q" else kt_sb
                    if dst[p] is None:
                        dst[p] = qk.tile([P, S], BF16, tag=tg,
                                         name=f"{tg}{p}")
                    psum_box = []

                    def mm(k):
                        if not psum_box:
                            # lazy psum alloc: keeps pchain-ring request
                            # order aligned with actual first-write order
                            psum_box.append(pch.tile(
                                [P, FREE], F32, tag="chain",
                                name=f"{which}ps{p}_{ih}"))
                        nc.tensor.matmul(
                            psum_box[0][:],
                            w_sb[p][:, k * P:(k + 1) * P],
                            x_sb[k][:, ih * FREE:(ih + 1) * FREE],
                            start=(k == 0), stop=(k == KT - 1))

                    def drain():
                        nc.vector.tensor_scalar(
                            out=dst[p][:, ih * FREE:(ih + 1) * FREE],
                            in0=psum_box[0][:], scalar1=b_sb[:, p:p + 1],
                            scalar2=None, op0=mybir.AluOpType.add)

                    return mm, drain

                attn_of = {}

                def scores_slot(p, jt):
                    h0, h1 = 2 * p, 2 * p + 1
                    psA = ps.tile([P, S], F32, tag="big", name=f"sps{h0}_{jt}")
                    psB = ps.tile([P, S], F32, tag="big", name=f"sps{h1}_{jt}")
                    for ih in range(NIH):
                        nc.tensor.matmul(
                            psA[:, ih * FREE:(ih + 1) * FREE],
                            kt_sb[p][0:DK, jt * P:(jt + 1) * P],
                            qt_sb[p][0:DK, ih * FREE:(ih + 1) * FREE],
                            start=True, stop=True)
                        nc.tensor.matmul(
                            psB[:, ih * FREE:(ih + 1) * FREE],
                            kt_sb[p][DK:P, jt * P:(jt + 1) * P],
                            qt_sb[p][DK:P, ih * FREE:(ih + 1) * FREE],
                            start=True, stop=True)
                    for h, pst in ((h0, psA), (h1, psB)):
                        a = att.tile([P, S], BF16, tag="attn",
                                     name=f"attn{h}_{jt}")
                        nc.scalar.activation(
                            a[:], pst[:], mybir.ActivationFunctionType.Exp,
                            scale=EXP_SCALE)
                        attn_of[(h, jt)] = a

                ctxt_sb = [ctxp.tile([P, S], BF16, tag=f"ctxt{et}",
                                     name=f"ctxt{et}") for et in range(ET)]
                ctx_chains = {}

                def ctx_mms(h, s):
                    """ctx-chain step s (0..3) for head h: j-tiles 2s, 2s+1."""
                    if s == 0:
                        ctx_chains[h] = [
                            pch.tile([P, FREE], F32, tag="chain",
                                     name=f"cps{h}_{ih}") for ih in range(NIH)]
                    chains = ctx_chains[h]
                    for ih in range(NIH):
                        for j2 in (2 * s, 2 * s + 1):
                            nc.tensor.matmul(
                                chains[ih][:],
                                vaug[j2][:, h, :],
                                attn_of[(h, j2)][:, ih * FREE:(ih + 1) * FREE],
                                start=(j2 == 0), stop=(j2 == ST - 1))

                do_norm = "nonorm" not in stage_set

                def ctx_norm(h):
                    chains = ctx_chains.pop(h)
                    et = h // 2
                    pr = slice((h % 2) * DK, (h % 2) * DK + DK)
                    for ih in range(NIH):
                        cps = chains[ih]
                        if do_norm:
                            rb = rbp.tile([DK, FREE], F32, tag="rb",
                                          name=f"rb{h}_{ih}")
                            nc.vector.reciprocal_approx_fast(
                                out=rb[:], in_=cps[0:DK, :])
                            nc.vector.tensor_tensor(
                                out=ctxt_sb[et][pr, ih * FREE:(ih + 1) * FREE],
                                in0=cps[DK:P, :], in1=rb[:],
                                op=mybir.AluOpType.mult)
                        else:
                            nc.vector.tensor_scalar(
                                out=ctxt_sb[et][pr,
                                                ih * FREE:(ih + 1) * FREE],
                                in0=cps[DK:P, :], scalar1=1.0,
                                scalar2=None, op0=mybir.AluOpType.mult)
                    for jt in range(ST):
                        attn_of.pop((h, jt))

                # ---- prologue: Q0/K0 chains, then V projection ----
                if do_qk:
                    for which in ("q", "k"):
                        for ih in range(NIH):
                            mm, drain = proj_chain(0, which, ih)
                            for k in range(KT):
                                mm(k)
                            drain()
                for st in range(ST if do_v else 0):
                    psum = ps.tile([P, D], F32, tag="big", name=f"vps{st}")
                    for eh in range(NIH):
                        for k in range(KT):
                            nc.tensor.matmul(
                                psum[:, eh * FREE:(eh + 1) * FREE],
                                xv_sb[k][:, st * P:(st + 1) * P],
                                wv_sb[k][:, eh * FREE:(eh + 1) * FREE],
                                start=(k == 0), stop=(k == KT - 1))
                    nc.vector.tensor_tensor(
                        out=vaug[st][:, :, DK:2 * DK],
                        in0=psum[:].rearrange("p (h c) -> p h c", h=H),
                        in1=bvb_sb[:].rearrange("p (h c) -> p h c", h=H),
                        op=mybir.AluOpType.add)

                # ---- pair loop ----
                for p in range(ET if do_scores else 0):
                    proj_parts = []
                    if p + 1 < ET:
                        for which in ("q", "k"):
                            for ih in range(NIH):
                                proj_parts.append(proj_chain(p + 1, which, ih))
                    for jt in range(ST):
                        scores_slot(p, jt)
                        if do_ctx and p >= 1:
                            hc = 2 * (p - 1) + (0 if jt < 4 else 1)
                            ctx_mms(hc, jt % 4)
                            if jt % 4 == 3:
                                ctx_norm(hc)
                        if proj_parts:
                            ci, phase = divmod(jt, 2)
                            mm, drain = proj_parts[ci]
                            for k in range(4 * phase, 4 * phase + 4):
                                mm(k)
                            if phase == 1:
                                drain()

                # ---- epilogue: ctx(7) overlapped with out-proj k<7 ----
                if not do_ctx:
                    o = outp.tile([P, D], F32, tag="o", name="o_stub")
                    nc.vector.memset(o[:], 0.0)
                    for st in range(ST):
                        nc.scalar.dma_start(
                            out=out_d[st * P:(st + 1) * P, :], in_=o[:])
                    return
                for s in range(4):
                    ctx_mms(H - 2, s)
                ctx_norm(H - 2)
                for s in range(4):
                    ctx_mms(H - 1, s)
                ctx_norm(H - 1)
                if do_out:
                    for st in range(ST):
                        psum = ps.tile([P, D], F32, tag="big", name=f"ops{st}")
                        for eh in range(NIH):
                            for k in range(KT):
                                nc.tensor.matmul(
                                    psum[:, eh * FREE:(eh + 1) * FREE],
                                    ctxt_sb[k][:, st * P:(st + 1) * P],
                                    wo_sb[k][:, eh * FREE:(eh + 1) * FREE],
                                    start=(k == 0), stop=(k == KT - 1))
                        o = outp.tile([P, D], F32, tag="o", name=f"o{st}")
                        nc.vector.tensor_tensor(out=o[:], in0=psum[:],
                                                in1=bob_sb[:],
                                                op=mybir.AluOpType.add)
                        nc.scalar.dma_start(
                            out=out_d[st * P:(st + 1) * P, :], in_=o[:])
                else:
                    o = outp.tile([P, D], F32, tag="o", name="o_stub")
                    nc.vector.memset(o[:], 0.0)
                    for st in range(ST):
                        nc.scalar.dma_start(
                            out=out_d[st * P:(st + 1) * P, :], in_=o[:])

            if repeat == 1:
                body()
            elif repeat < 0:
                for _ in range(-repeat):
                    body()
            else:
                with tc.For_i(0, repeat, 1) as _:
                    body()

    nc.compile()
    return nc


_NC_CACHE: dict = {}


def get_nc(repeat: int = 1):
    if repeat not in _NC_CACHE:
        _NC_CACHE[repeat] = build_nc(repeat)
    return _NC_CACHE[repeat]


def _etmajor(w_t: np.ndarray) -> np.ndarray:
    """[D, E] -> e-tile-major layout: out[et*P+p, k*P+e] = w_t[k*P+p, et*P+e]."""
    return np.ascontiguousarray(
        w_t.reshape(KT, P, ET, P).transpose(2, 1, 0, 3).reshape(D, D))


def make_in_maps(query, key_, value, w_q, b_q, w_k, b_k, w_v, b_v, w_o, b_o):
    wq_t = np.asarray(w_q, np.float32).T
    wk_t = np.asarray(w_k, np.float32).T
    shared = {
        "wq_e": _etmajor(wq_t).astype(BF),
        "wk_e": _etmajor(wk_t).astype(BF),
        "wv_t": np.ascontiguousarray(np.asarray(w_v, np.float32).T).astype(BF),
        "wo_t": np.ascontiguousarray(np.asarray(w_o, np.float32).T).astype(BF),
        "bq_r": np.ascontiguousarray(
            np.asarray(b_q, np.float32).reshape(ET, P).T),
        "bk_r": np.ascontiguousarray(
            np.asarray(b_k, np.float32).reshape(ET, P).T),
        "bvb": np.ascontiguousarray(
            np.tile(np.asarray(b_v, np.float32)[None, :], (P, 1))).astype(BF),
        "bob": np.ascontiguousarray(
            np.tile(np.asarray(b_o, np.float32)[None, :], (P, 1))).astype(BF),
    }
    q = np.asarray(query, np.float32)
    k = np.asarray(key_, np.float32)
    v = np.asarray(value, np.float32)
    in_maps = []
    for b in range(B):
        m = dict(shared)
        m["xq_t"] = np.ascontiguousarray(q[b].T).astype(BF)
        m["xk_t"] = np.ascontiguousarray(k[b].T).astype(BF)
        m["xv_t"] = np.ascontiguousarray(v[b].T).astype(BF)
        in_maps.append(m)
    return in_maps


def run(in_maps, repeat: int = 1):
    nc = get_nc(repeat)
    res = run_bass_kernel_spmd(nc, in_maps, list(range(N_CORES)))
    return np.stack([np.asarray(res.results[i]["out"], np.float32)
                     for i in range(B)])


def kernel(query, key_, value, w_q, b_q, w_k, b_k, w_v, b_v, w_o, b_o):
    in_maps = make_in_maps(query, key_, value, w_q, b_q, w_k, b_k,
                           w_v, b_v, w_o, b_o)
    return run(in_maps, repeat=1)
